# revision 46
# baseline (speedup 1.0000x reference)
"""Multi-head attention (B=2, T=2048, D=1024, 16 heads) on 8 TRN2 NeuronCores.

Sharding: tensor-parallel over heads (2 heads/core). Each core computes
Q/K/V projections for its 2 heads (full sequence), causal attention in the
S^T = K @ Q^T form, and a partial output projection; the host sums the 8
partial outputs.

v7: fully interleaved schedule. Projection work for chunk-pair jp+1 is
split into units (one PSUM accumulation group of 8 matmuls + evacuation,
or one V-transpose) that are popped between attention blocks of chunks
2jp/2jp+1, so the tensor engine never idles during the ACT-bound exp
chain and the HAM clock gate stays at full speed. PSUM: one shared
2-slot ring for {proj accumulators, o-proj outputs, V transposes} (the
consumer of each slot is a fast evacuation copy), 4 banks for the S
double-buffer, 2 for the AV accumulators. bf16 everywhere; causal
masking via gpsimd affine_select on the diagonal blocks (processed
first, so the select latency hides in AV-chain slack); softmax
denominators via a ones-column in V + reciprocal_approx_fast, with the
per-head reciprocal rows broadcast across partitions through a DRAM
bounce (DMA stride-0 reads). A dummy-matmul stream during the prologue
DMA wait pre-warms the HAM clock gate. Measured: 187-196us HW exec
(mean ~191, +/-7us HAM-phase variance), rel err 4.6e-3 (baseline:
293-347us, rel err 2.2e-4; tolerance 2e-2).
"""

import sys

sys.path.insert(0, "/opt/trn_rl_repo")

import numpy as np

B, T, D = 2, 2048, 1024
NCORES = 8
DV = 128  # head dims per core (2 heads x 64)
DH = 64
BT = B * T
CH = 512  # tq chunk width
NCH = BT // CH  # 8 global chunks
NCH_B = T // CH  # 4 chunks per batch
TK = 128  # tk tile
NTK = T // TK  # 16 tiles per batch
ND = D // 128  # 8 contraction tiles
DVA = DH + 1  # V columns incl ones column (for softmax denominator)

_cache = {}


def _build(debug=False):
    import concourse.bacc as bacc
    import concourse.mybir as mybir
    import concourse.tile as tile

    F32 = mybir.dt.float32
    BF16 = mybir.dt.bfloat16
    EXP = mybir.ActivationFunctionType.Exp
    MULT = mybir.AluOpType.mult
    from concourse.masks import make_identity

    nc = bacc.Bacc("TRN2", target_bir_lowering=False, debug=False,
                   num_devices=NCORES)

    xt_d = nc.dram_tensor("xt", [D, BT], BF16, kind="ExternalInput").ap()
    wq_d = nc.dram_tensor("wq", [128, D], BF16, kind="ExternalInput").ap()
    wk_d = nc.dram_tensor("wk", [128, D], BF16, kind="ExternalInput").ap()
    wv_d = nc.dram_tensor("wv", [128, D], BF16, kind="ExternalInput").ap()
    wo_d = nc.dram_tensor("wo", [128, D], BF16, kind="ExternalInput").ap()
    o_d = nc.dram_tensor("o", [BT, D], BF16, kind="ExternalOutput").ap()

    NU = B * NTK * 2  # 64 per-head V blocks

    with tile.TileContext(nc) as tc:
        with tc.tile_pool(name="consts", bufs=1) as consts, \
             tc.tile_pool(name="perm", bufs=1) as perm, \
             tc.tile_pool(name="vtf_pool", bufs=2) as vtfp, \
             tc.tile_pool(name="p_pool", bufs=6) as ppool, \
             tc.tile_pool(name="avc_pool", bufs=2) as avcp, \
             tc.tile_pool(name="rec_pool", bufs=2) as recp, \
             tc.tile_pool(name="outT_pool", bufs=2) as outTp, \
             tc.tile_pool(name="osb_pool", bufs=3) as obp, \
             tc.tile_pool(name="acc_ps", bufs=2, space="PSUM") as accp, \
             tc.tile_pool(name="s_ps", bufs=2, space="PSUM") as spsp, \
             tc.tile_pool(name="av_ps", bufs=1, space="PSUM") as avp, \
             tc.tile_pool(name="dram_pool", bufs=2, space="DRAM") as drp:
            wq_sb = consts.tile([128, D], BF16, name="wq_sb")
            wk_sb = consts.tile([128, D], BF16, name="wk_sb")
            wv_sb = consts.tile([128, D], BF16, name="wv_sb")
            wo_sb = consts.tile([128, D], BF16, name="wo_sb")
            ident = consts.tile([128, 128], BF16, name="ident")
            make_identity(nc, ident[:])

            qT = perm.tile([128, BT], BF16, name="qT")
            kT = perm.tile([128, BT], BF16, name="kT")
            # V blocks, t-major with ones column: per (b, tile, head) a
            # [128(tk), 65] block at free offset u*65, u = (b*NTK+i)*2+h
            vsb = perm.tile([128, NU * DVA], BF16, name="vsb")
            nc.gpsimd.memset(
                vsb[:].rearrange("p (u c) -> p u c", c=DVA)[:, :, DH:DVA], 1.0)

            # ---------------- projection work units ----------------
            units = []
            xall = []

            def emit_proj_pair(jp):
                j0 = 2 * jp
                xts = {}

                if jp == 0:
                    def u_dma():
                        for d in range(ND):
                            xa = perm.tile([128, BT], BF16,
                                           name=f"xall{d}")
                            eng = nc.scalar if d % 2 else nc.sync
                            eng.dma_start(
                                xa[:], xt_d[d * 128:(d + 1) * 128, :])
                            xall.append(xa)
                            ws = slice(d * 128, (d + 1) * 128)
                            nc.scalar.dma_start(wv_sb[:, ws], wv_d[:, ws])
                            nc.scalar.dma_start(wq_sb[:, ws], wq_d[:, ws])
                            nc.scalar.dma_start(wk_sb[:, ws], wk_d[:, ws])
                    units.append(u_dma)

                for half in (0, 1):
                    for nm in ("v", "q", "k"):
                        def u_group(nm=nm, half=half, jp=jp, j0=j0, xts=xts):
                            j = j0 + half
                            a = accp.tile([128, CH], F32, tag="acc",
                                          name=f"a_{nm}{j}")
                            w_sb = {"q": wq_sb, "k": wk_sb, "v": wv_sb}[nm]
                            c0 = (j0 + half) * CH
                            for d in range(ND):
                                ws = slice(d * 128, (d + 1) * 128)
                                nc.tensor.matmul(
                                    a[:], w_sb[:, ws],
                                    xall[d][:, c0:c0 + CH],
                                    start=d == 0, stop=d == ND - 1)
                            cs = slice(j * CH, (j + 1) * CH)
                            if nm == "q":
                                nc.vector.tensor_copy(qT[:, cs], a[:])
                            elif nm == "k":
                                nc.scalar.copy(kT[:, cs], a[:])
                            else:
                                vtf = vtfp.tile([128, CH], BF16, tag="vtf",
                                                name=f"vtf{j}")
                                nc.vector.tensor_copy(vtf[:], a[:])
                                xts[("vtf", half)] = vtf
                        units.append(u_group)
                    if half == 0 and jp == 0:
                        def u_wo():
                            nc.scalar.dma_start(wo_sb[:], wo_d[:])
                        units.append(u_wo)

                    for tt in range(4):
                        def u_vt(half=half, tt=tt, j0=j0, xts=xts):
                            j = j0 + half
                            tglob = 4 * j + tt
                            bb, ii = tglob // NTK, tglob % NTK
                            u0 = (bb * NTK + ii) * 2
                            vtf = xts[("vtf", half)]
                            vt_ps = accp.tile([128, 128], BF16, tag="acc",
                                              name=f"vt{tglob}")
                            nc.tensor.transpose(
                                vt_ps[:], vtf[:, tt * 128:(tt + 1) * 128],
                                ident[:])
                            dst = vsb[:, u0 * DVA:(u0 + 2) * DVA].rearrange(
                                "p (h c) -> p h c", c=DVA)[:, :, 0:DH]
                            nc.vector.tensor_copy(
                                dst, vt_ps[:].rearrange(
                                    "p (h c) -> p h c", c=DH))
                        units.append(u_vt)

            # ---------------- o-proj deferral ----------------
            deferred = []

            def emit_oproj(b, jj, outT):
                for tt in range(4):
                    osb = obp.tile([128, D], BF16, tag="osb",
                                   name=f"osb{b}_{jj}_{tt}")
                    for half in (0, 1):
                        def step(tt=tt, half=half, b=b, jj=jj,
                                 outT=outT, osb=osb, alt=False):
                            op = accp.tile([128, CH], F32, tag="acc",
                                           name=f"op{b}_{jj}_{tt}_{half}")
                            ts = slice(tt * 128, (tt + 1) * 128)
                            hs = slice(half * CH, (half + 1) * CH)
                            nc.tensor.matmul(op[:], outT[:, ts],
                                             wo_sb[:, hs],
                                             start=True, stop=True)
                            if alt:
                                nc.scalar.copy(osb[:, hs], op[:])
                            else:
                                nc.vector.tensor_copy(osb[:, hs], op[:])
                            if half == 1:
                                r0 = b * T + jj * CH + tt * 128
                                nc.sync.dma_start(o_d[r0:r0 + 128, :],
                                                  osb[:])
                        deferred.append(step)

            def pop_work():
                if units:
                    units.pop(0)()
                    if len(units) > 8:
                        units.pop(0)()
                elif deferred:
                    deferred.pop(0)()

            # ---------------- attention ----------------
            def attention_chunk(b, jj, last=False):
                # diagonal blocks first (r=0 full-width leads the av
                # accumulation; select latency absorbs into AV slack)
                kept = list(range(4 * jj, 4 * jj + 4)) + list(range(4 * jj))
                av0 = avp.tile([DVA, CH], F32, tag="av0",
                               name=f"av0_{b}_{jj}")
                av1 = avp.tile([DVA, CH], F32, tag="av1",
                               name=f"av1_{b}_{jj}")
                tq0 = (b * NCH_B + jj) * CH
                pend = None

                def emit_av(i, p, n0):
                    st = i == kept[0]
                    sp = i == kept[-1]
                    u0 = (b * NTK + i) * 2
                    nc.tensor.matmul(
                        av0[:, n0:CH], vsb[:, u0 * DVA:u0 * DVA + DVA],
                        p[:, 0, n0:CH], start=st, stop=sp)
                    nc.tensor.matmul(
                        av1[:, n0:CH], vsb[:, (u0 + 1) * DVA:(u0 + 2) * DVA],
                        p[:, 1, n0:CH], start=st, stop=sp)

                for i in kept:
                    r = i - 4 * jj  # diagonal sub-block index (>=0: diag)
                    n0 = 128 * r if r > 0 else 0  # first valid tq column
                    ks = slice((b * NTK + i) * TK, (b * NTK + i + 1) * TK)
                    sps = spsp.tile([128, 2, CH], F32, tag="sps",
                                    name=f"sps{b}_{jj}_{i}")
                    nc.tensor.matmul(sps[:, 0, n0:CH], kT[0:64, ks],
                                     qT[0:64, tq0 + n0:tq0 + CH],
                                     start=True, stop=True)
                    nc.tensor.matmul(sps[:, 1, n0:CH], kT[64:128, ks],
                                     qT[64:128, tq0 + n0:tq0 + CH],
                                     start=True, stop=True)
                    p = ppool.tile([128, 2, CH], BF16, tag="p",
                                   name=f"p{b}_{jj}_{i}")
                    nc.scalar.activation(p[:, :, n0:CH], sps[:, :, n0:CH],
                                         EXP)
                    if r >= 0:
                        # zero the strict upper triangle of the diagonal
                        # [128,128] sub-block: keep iff tk(partition) <= tq
                        nc.gpsimd.affine_select(
                            out=p[:, :, n0:n0 + 128],
                            in_=p[:, :, n0:n0 + 128],
                            compare_op=mybir.AluOpType.is_ge,
                            fill=0.0,
                            base=0,
                            pattern=[[0, 2], [1, 128]],
                            channel_multiplier=-1,
                        )
                    if pend is not None:
                        emit_av(*pend)
                    if not last:
                        pop_work()
                    pend = (i, p, n0)
                emit_av(*pend)

                # evacuate av banks; row DH holds the softmax denominators
                avc = avcp.tile([128, CH], F32, tag="avc",
                                name=f"avc_{b}_{jj}")
                srow = avcp.tile([33, CH], F32, tag="srow",
                                 name=f"srow_{b}_{jj}")
                nc.scalar.copy(avc[64:128, :], av1[0:DH, :])
                nc.scalar.copy(srow[32:33, :], av1[DH:DVA, :])
                nc.vector.tensor_copy(avc[0:64, :], av0[0:DH, :])
                nc.vector.tensor_copy(srow[0:1, :], av0[DH:DVA, :])
                rec = recp.tile([33, CH], F32, tag="rec", name=f"rec{b}_{jj}")
                # one instr covers both sums rows (0 and 32); rows 1..31
                # are don't-care garbage
                nc.vector.reciprocal_approx_fast(rec[0:33, :], srow[0:33, :])
                dr = drp.tile([2, CH], F32, tag="dr", name=f"dr_{b}_{jj}")
                nc.sync.dma_start(dr[0:1, :], rec[0:1, :])
                nc.sync.dma_start(dr[1:2, :], rec[32:33, :])
                rbc = recp.tile([128, CH], F32, tag="rbc", name=f"rbc{b}_{jj}")
                nc.sync.dma_start(rbc[0:64, :],
                                  dr[0:1, :].broadcast_to([64, CH]))
                nc.sync.dma_start(rbc[64:128, :],
                                  dr[1:2, :].broadcast_to([64, CH]))
                outT = outTp.tile([128, CH], BF16, tag="outT",
                                  name=f"outT{b}_{jj}")
                nc.gpsimd.tensor_tensor(out=outT[0:64, :], in0=avc[0:64, :],
                                        in1=rbc[0:64, :], op=MULT)
                nc.gpsimd.tensor_tensor(out=outT[64:128, :],
                                        in0=avc[64:128, :],
                                        in1=rbc[64:128, :], op=MULT)
                emit_oproj(b, jj, outT)

            # ---------------- main schedule ----------------
            # HAM warmup: the prologue waits ~10us on DMA with the PE idle,
            # so the first real matmuls would run at the cold 1.2GHz clock.
            # A stream of dummy matmuls during the wait flips the clock gate
            # to 8/8 before real work starts.
            warm_sc = consts.tile([128, 128], BF16, name="warm_sc")
            wps = accp.tile([128, 128], F32, tag="acc", name="warm_ps")
            for wi in range(40):
                nc.tensor.matmul(wps[:], ident[:], ident[:],
                                 start=wi == 0, stop=wi == 39)
            nc.vector.tensor_copy(warm_sc[:], wps[:])
            emit_proj_pair(0)
            for _ in range(9):  # prologue: pair 0 first half only
                units.pop(0)()
            for c in range(NCH):
                b, jj = c // NCH_B, c % NCH_B
                if c % 2 == 0 and c // 2 + 1 < NCH // 2:
                    emit_proj_pair(c // 2 + 1)
                attention_chunk(b, jj, last=c == NCH - 1)
                if c % 2 == 1:
                    while units:  # chunk c+1 needs its pair complete
                        units.pop(0)()
            fi = 0
            while deferred:
                deferred.pop(0)(alt=fi % 2 == 1)
                fi += 1

    nc.compile()
    return nc


def kernel(x, Wq, Wk, Wv, Wo, attn_mask):
    import concourse.bass_utils as _bu
    import ml_dtypes
    run_bass_kernel_spmd = _bu.run_bass_kernel_spmd
    BF = ml_dtypes.bfloat16

    x = np.asarray(x, dtype=np.float32)
    Wq = np.asarray(Wq, dtype=np.float32)
    Wk = np.asarray(Wk, dtype=np.float32)
    Wv = np.asarray(Wv, dtype=np.float32)
    Wo = np.asarray(Wo, dtype=np.float32)

    xT = np.ascontiguousarray(x.reshape(BT, D).T).astype(BF)

    if "nc" not in _cache:
        _cache["nc"] = _build()
    nc = _cache["nc"]

    in_maps = []
    for c in range(NCORES):
        rows = slice(c * DV, (c + 1) * DV)

        def wlayout(W, scale=1.0):
            Wc = W[rows, :]  # [128, D]
            return np.ascontiguousarray(
                (Wc.T.reshape(ND, 128, 128).transpose(1, 0, 2)
                 .reshape(128, D) * scale)).astype(BF)

        wo_dev = np.ascontiguousarray(Wo[:, rows].T).astype(BF)
        in_maps.append({
            "xt": xT,
            "wq": wlayout(Wq, 0.125),
            "wk": wlayout(Wk),
            "wv": wlayout(Wv),
            "wo": wo_dev,
        })

    res = run_bass_kernel_spmd(nc, in_maps, core_ids=list(range(NCORES)))
    _cache["last_res"] = res
    out = np.zeros((BT, D), dtype=np.float32)
    for c in range(NCORES):
        out += np.asarray(res.results[c]["o"]).astype(np.float32)
    return out.reshape(B, T, D)


# revision 47
# speedup vs baseline: 1.0503x; 1.0503x over previous
"""Multi-head attention (B=2, T=2048, D=1024, 16 heads) on 8 TRN2 NeuronCores.

Sharding: tensor-parallel over heads (2 heads/core). Each core computes
Q/K/V projections for its 2 heads (full sequence), causal attention in the
S^T = K @ Q^T form, and a partial output projection; the host sums the 8
partial outputs.

v7: fully interleaved schedule. Projection work for chunk-pair jp+1 is
split into units (one PSUM accumulation group of 8 matmuls + evacuation,
or one V-transpose) that are popped between attention blocks of chunks
2jp/2jp+1, so the tensor engine never idles during the ACT-bound exp
chain and the HAM clock gate stays at full speed. PSUM: one shared
2-slot ring for {proj accumulators, o-proj outputs, V transposes} (the
consumer of each slot is a fast evacuation copy), 4 banks for the S
double-buffer, 2 for the AV accumulators. bf16 everywhere; causal
masking via gpsimd affine_select on the diagonal blocks (processed
first, so the select latency hides in AV-chain slack); softmax
denominators via a ones-column in V + reciprocal_approx_fast, with the
per-head reciprocal rows broadcast across partitions through a DRAM
bounce (DMA stride-0 reads). A dummy-matmul stream during the prologue
DMA wait pre-warms the HAM clock gate. Measured: 187-196us HW exec
(mean ~191, +/-7us HAM-phase variance), rel err 4.6e-3 (baseline:
293-347us, rel err 2.2e-4; tolerance 2e-2).
"""

import sys

sys.path.insert(0, "/opt/trn_rl_repo")

import numpy as np

B, T, D = 2, 2048, 1024
NCORES = 8
DV = 128  # head dims per core (2 heads x 64)
DH = 64
BT = B * T
CH = 512  # tq chunk width
NCH = BT // CH  # 8 global chunks
NCH_B = T // CH  # 4 chunks per batch
TK = 128  # tk tile
NTK = T // TK  # 16 tiles per batch
ND = D // 128  # 8 contraction tiles
DVA = DH + 1  # V columns incl ones column (for softmax denominator)

_cache = {}


def _build(debug=False):
    import concourse.bacc as bacc
    import concourse.mybir as mybir
    import concourse.tile as tile

    F32 = mybir.dt.float32
    BF16 = mybir.dt.bfloat16
    EXP = mybir.ActivationFunctionType.Exp
    MULT = mybir.AluOpType.mult
    from concourse.masks import make_identity

    nc = bacc.Bacc("TRN2", target_bir_lowering=False, debug=False,
                   num_devices=NCORES)

    xt_d = nc.dram_tensor("xt", [D, BT], BF16, kind="ExternalInput").ap()
    wq_d = nc.dram_tensor("wq", [128, D], BF16, kind="ExternalInput").ap()
    wk_d = nc.dram_tensor("wk", [128, D], BF16, kind="ExternalInput").ap()
    wv_d = nc.dram_tensor("wv", [128, D], BF16, kind="ExternalInput").ap()
    wo_d = nc.dram_tensor("wo", [128, D], BF16, kind="ExternalInput").ap()
    o_d = nc.dram_tensor("o", [BT, D], BF16, kind="ExternalOutput").ap()

    NU = B * NTK * 2  # 64 per-head V blocks

    with tile.TileContext(nc) as tc:
        with tc.tile_pool(name="consts", bufs=1) as consts, \
             tc.tile_pool(name="perm", bufs=1) as perm, \
             tc.tile_pool(name="xt_pool", bufs=9) as xtp, \
             tc.tile_pool(name="vtf_pool", bufs=2) as vtfp, \
             tc.tile_pool(name="p_pool", bufs=6) as ppool, \
             tc.tile_pool(name="avc_pool", bufs=2) as avcp, \
             tc.tile_pool(name="rec_pool", bufs=2) as recp, \
             tc.tile_pool(name="outT_pool", bufs=2) as outTp, \
             tc.tile_pool(name="osb_pool", bufs=3) as obp, \
             tc.tile_pool(name="acc_ps", bufs=2, space="PSUM") as accp, \
             tc.tile_pool(name="s_ps", bufs=2, space="PSUM") as spsp, \
             tc.tile_pool(name="av_ps", bufs=1, space="PSUM") as avp, \
             tc.tile_pool(name="dram_pool", bufs=2, space="DRAM") as drp:
            wq_sb = consts.tile([128, D], BF16, name="wq_sb")
            wk_sb = consts.tile([128, D], BF16, name="wk_sb")
            wv_sb = consts.tile([128, D], BF16, name="wv_sb")
            wo_sb = consts.tile([128, D], BF16, name="wo_sb")
            ident = consts.tile([128, 128], BF16, name="ident")
            make_identity(nc, ident[:])

            qT = perm.tile([128, BT], BF16, name="qT")
            kT = perm.tile([128, BT], BF16, name="kT")
            # V blocks, t-major with ones column: per (b, tile, head) a
            # [128(tk), 65] block at free offset u*65, u = (b*NTK+i)*2+h
            vsb = perm.tile([128, NU * DVA], BF16, name="vsb")
            nc.gpsimd.memset(
                vsb[:].rearrange("p (u c) -> p u c", c=DVA)[:, :, DH:DVA], 1.0)

            # ---------------- projection work units ----------------
            units = []

            def emit_proj_pair(jp):
                j0 = 2 * jp
                xts = {}

                def u_dma(jp=jp, j0=j0, xts=xts):
                    for d in range(ND):
                        xt = xtp.tile([128, 2 * CH], BF16, tag="xt",
                                      name=f"xt{jp}_{d}")
                        eng = nc.scalar if (jp == 0 and d % 2) else nc.sync
                        eng.dma_start(
                            xt[:], xt_d[d * 128:(d + 1) * 128,
                                        j0 * CH:(j0 + 2) * CH])
                        xts[d] = xt
                        if jp == 0:
                            ws = slice(d * 128, (d + 1) * 128)
                            nc.scalar.dma_start(wv_sb[:, ws], wv_d[:, ws])
                            nc.scalar.dma_start(wq_sb[:, ws], wq_d[:, ws])
                            nc.scalar.dma_start(wk_sb[:, ws], wk_d[:, ws])
                units.append(u_dma)

                for half in (0, 1):
                    for nm in ("v", "q", "k"):
                        def u_group(nm=nm, half=half, jp=jp, j0=j0, xts=xts):
                            j = j0 + half
                            a = accp.tile([128, CH], F32, tag="acc",
                                          name=f"a_{nm}{j}")
                            w_sb = {"q": wq_sb, "k": wk_sb, "v": wv_sb}[nm]
                            for d in range(ND):
                                ws = slice(d * 128, (d + 1) * 128)
                                nc.tensor.matmul(
                                    a[:], w_sb[:, ws],
                                    xts[d][:, half * CH:(half + 1) * CH],
                                    start=d == 0, stop=d == ND - 1)
                            cs = slice(j * CH, (j + 1) * CH)
                            if nm == "q":
                                nc.vector.tensor_copy(qT[:, cs], a[:])
                            elif nm == "k":
                                nc.scalar.copy(kT[:, cs], a[:])
                            else:
                                vtf = vtfp.tile([128, CH], BF16, tag="vtf",
                                                name=f"vtf{j}")
                                nc.vector.tensor_copy(vtf[:], a[:])
                                xts[("vtf", half)] = vtf
                        units.append(u_group)
                    if half == 0 and jp == 0:
                        def u_wo():
                            nc.scalar.dma_start(wo_sb[:], wo_d[:])
                        units.append(u_wo)

                    for tt in range(4):
                        def u_vt(half=half, tt=tt, j0=j0, xts=xts):
                            j = j0 + half
                            tglob = 4 * j + tt
                            bb, ii = tglob // NTK, tglob % NTK
                            u0 = (bb * NTK + ii) * 2
                            vtf = xts[("vtf", half)]
                            vt_ps = accp.tile([128, 128], BF16, tag="acc",
                                              name=f"vt{tglob}")
                            nc.tensor.transpose(
                                vt_ps[:], vtf[:, tt * 128:(tt + 1) * 128],
                                ident[:])
                            dst = vsb[:, u0 * DVA:(u0 + 2) * DVA].rearrange(
                                "p (h c) -> p h c", c=DVA)[:, :, 0:DH]
                            nc.vector.tensor_copy(
                                dst, vt_ps[:].rearrange(
                                    "p (h c) -> p h c", c=DH))
                        units.append(u_vt)

            # ---------------- o-proj deferral ----------------
            deferred = []

            def emit_oproj(b, jj, outT):
                for tt in range(4):
                    osb = obp.tile([128, D], BF16, tag="osb",
                                   name=f"osb{b}_{jj}_{tt}")
                    for half in (0, 1):
                        def step(tt=tt, half=half, b=b, jj=jj,
                                 outT=outT, osb=osb, alt=False):
                            op = accp.tile([128, CH], F32, tag="acc",
                                           name=f"op{b}_{jj}_{tt}_{half}")
                            ts = slice(tt * 128, (tt + 1) * 128)
                            hs = slice(half * CH, (half + 1) * CH)
                            nc.tensor.matmul(op[:], outT[:, ts],
                                             wo_sb[:, hs],
                                             start=True, stop=True)
                            if alt:
                                nc.scalar.copy(osb[:, hs], op[:])
                            else:
                                nc.vector.tensor_copy(osb[:, hs], op[:])
                            if half == 1:
                                r0 = b * T + jj * CH + tt * 128
                                nc.sync.dma_start(o_d[r0:r0 + 128, :],
                                                  osb[:])
                        deferred.append(step)

            def pop_work():
                if units:
                    units.pop(0)()
                    if len(units) > 8:
                        units.pop(0)()
                elif deferred:
                    deferred.pop(0)()

            # ---------------- attention ----------------
            def attention_chunk(b, jj, last=False):
                # diagonal blocks first (r=0 full-width leads the av
                # accumulation; select latency absorbs into AV slack)
                kept = list(range(4 * jj, 4 * jj + 4)) + list(range(4 * jj))
                av0 = avp.tile([DVA, CH], F32, tag="av0",
                               name=f"av0_{b}_{jj}")
                av1 = avp.tile([DVA, CH], F32, tag="av1",
                               name=f"av1_{b}_{jj}")
                tq0 = (b * NCH_B + jj) * CH
                pend = None

                def emit_av(i, p, n0):
                    st = i == kept[0]
                    sp = i == kept[-1]
                    u0 = (b * NTK + i) * 2
                    nc.tensor.matmul(
                        av0[:, n0:CH], vsb[:, u0 * DVA:u0 * DVA + DVA],
                        p[:, 0, n0:CH], start=st, stop=sp)
                    nc.tensor.matmul(
                        av1[:, n0:CH], vsb[:, (u0 + 1) * DVA:(u0 + 2) * DVA],
                        p[:, 1, n0:CH], start=st, stop=sp)

                for i in kept:
                    r = i - 4 * jj  # diagonal sub-block index (>=0: diag)
                    n0 = 128 * r if r > 0 else 0  # first valid tq column
                    ks = slice((b * NTK + i) * TK, (b * NTK + i + 1) * TK)
                    sps = spsp.tile([128, 2, CH], F32, tag="sps",
                                    name=f"sps{b}_{jj}_{i}")
                    nc.tensor.matmul(sps[:, 0, n0:CH], kT[0:64, ks],
                                     qT[0:64, tq0 + n0:tq0 + CH],
                                     start=True, stop=True)
                    nc.tensor.matmul(sps[:, 1, n0:CH], kT[64:128, ks],
                                     qT[64:128, tq0 + n0:tq0 + CH],
                                     start=True, stop=True)
                    p = ppool.tile([128, 2, CH], BF16, tag="p",
                                   name=f"p{b}_{jj}_{i}")
                    nc.scalar.activation(p[:, :, n0:CH], sps[:, :, n0:CH],
                                         EXP)
                    if r >= 0:
                        # zero the strict upper triangle of the diagonal
                        # [128,128] sub-block: keep iff tk(partition) <= tq
                        nc.gpsimd.affine_select(
                            out=p[:, :, n0:n0 + 128],
                            in_=p[:, :, n0:n0 + 128],
                            compare_op=mybir.AluOpType.is_ge,
                            fill=0.0,
                            base=0,
                            pattern=[[0, 2], [1, 128]],
                            channel_multiplier=-1,
                        )
                    if pend is not None:
                        emit_av(*pend)
                    if not last:
                        pop_work()
                    pend = (i, p, n0)
                emit_av(*pend)

                # evacuate av banks; row DH holds the softmax denominators
                avc = avcp.tile([128, CH], F32, tag="avc",
                                name=f"avc_{b}_{jj}")
                srow = avcp.tile([33, CH], F32, tag="srow",
                                 name=f"srow_{b}_{jj}")
                nc.scalar.copy(avc[64:128, :], av1[0:DH, :])
                nc.scalar.copy(srow[32:33, :], av1[DH:DVA, :])
                nc.vector.tensor_copy(avc[0:64, :], av0[0:DH, :])
                nc.vector.tensor_copy(srow[0:1, :], av0[DH:DVA, :])
                rec = recp.tile([33, CH], F32, tag="rec", name=f"rec{b}_{jj}")
                # one instr covers both sums rows (0 and 32); rows 1..31
                # are don't-care garbage
                nc.vector.reciprocal_approx_fast(rec[0:33, :], srow[0:33, :])
                dr = drp.tile([2, CH], F32, tag="dr", name=f"dr_{b}_{jj}")
                nc.sync.dma_start(dr[0:1, :], rec[0:1, :])
                nc.sync.dma_start(dr[1:2, :], rec[32:33, :])
                rbc = recp.tile([128, CH], F32, tag="rbc", name=f"rbc{b}_{jj}")
                nc.sync.dma_start(rbc[0:64, :],
                                  dr[0:1, :].broadcast_to([64, CH]))
                nc.sync.dma_start(rbc[64:128, :],
                                  dr[1:2, :].broadcast_to([64, CH]))
                outT = outTp.tile([128, CH], BF16, tag="outT",
                                  name=f"outT{b}_{jj}")
                nc.gpsimd.tensor_tensor(out=outT[0:64, :], in0=avc[0:64, :],
                                        in1=rbc[0:64, :], op=MULT)
                nc.gpsimd.tensor_tensor(out=outT[64:128, :],
                                        in0=avc[64:128, :],
                                        in1=rbc[64:128, :], op=MULT)
                emit_oproj(b, jj, outT)

            # ---------------- main schedule ----------------
            # HAM warmup: the prologue waits ~10us on DMA with the PE idle,
            # so the first real matmuls would run at the cold 1.2GHz clock.
            # A stream of dummy matmuls during the wait flips the clock gate
            # to 8/8 before real work starts.
            warm_sc = consts.tile([128, 128], BF16, name="warm_sc")
            wps = accp.tile([128, 128], F32, tag="acc", name="warm_ps")
            for wi in range(40):
                nc.tensor.matmul(wps[:], ident[:], ident[:],
                                 start=wi == 0, stop=wi == 39)
            nc.vector.tensor_copy(warm_sc[:], wps[:])
            emit_proj_pair(0)
            for _ in range(9):  # prologue: pair 0 first half only
                units.pop(0)()
            for c in range(NCH):
                b, jj = c // NCH_B, c % NCH_B
                if c % 2 == 0 and c // 2 + 1 < NCH // 2:
                    emit_proj_pair(c // 2 + 1)
                attention_chunk(b, jj, last=c == NCH - 1)
                if c % 2 == 1:
                    while units:  # chunk c+1 needs its pair complete
                        units.pop(0)()
            fi = 0
            while deferred:
                deferred.pop(0)(alt=fi % 2 == 1)
                fi += 1

    nc.compile()
    return nc


def kernel(x, Wq, Wk, Wv, Wo, attn_mask):
    import concourse.bass_utils as _bu
    import ml_dtypes
    run_bass_kernel_spmd = _bu.run_bass_kernel_spmd
    BF = ml_dtypes.bfloat16

    x = np.asarray(x, dtype=np.float32)
    Wq = np.asarray(Wq, dtype=np.float32)
    Wk = np.asarray(Wk, dtype=np.float32)
    Wv = np.asarray(Wv, dtype=np.float32)
    Wo = np.asarray(Wo, dtype=np.float32)

    xT = np.ascontiguousarray(x.reshape(BT, D).T).astype(BF)

    if "nc" not in _cache:
        _cache["nc"] = _build()
    nc = _cache["nc"]

    in_maps = []
    for c in range(NCORES):
        rows = slice(c * DV, (c + 1) * DV)

        def wlayout(W, scale=1.0):
            Wc = W[rows, :]  # [128, D]
            return np.ascontiguousarray(
                (Wc.T.reshape(ND, 128, 128).transpose(1, 0, 2)
                 .reshape(128, D) * scale)).astype(BF)

        wo_dev = np.ascontiguousarray(Wo[:, rows].T).astype(BF)
        in_maps.append({
            "xt": xT,
            "wq": wlayout(Wq, 0.125),
            "wk": wlayout(Wk),
            "wv": wlayout(Wv),
            "wo": wo_dev,
        })

    res = run_bass_kernel_spmd(nc, in_maps, core_ids=list(range(NCORES)))
    _cache["last_res"] = res
    out = np.zeros((BT, D), dtype=np.float32)
    for c in range(NCORES):
        out += np.asarray(res.results[c]["o"]).astype(np.float32)
    return out.reshape(B, T, D)
